# revision 3
# baseline (speedup 1.0000x reference)
"""GCN layer kernel for Trainium2, 8-core row-parallel.

Computes out = (adj * mask + I) @ (x @ W^T) for N=8192, C_in=C_out=128.

Sharding: adj/mask row-blocks of 1024 across 8 cores; x, W replicated.
v3 (host-transposed A stream):
  - each core's adj/mask row-slice is uploaded TRANSPOSED (adjT/maskT =
    [N, R] f32, a pure host-side layout choice).  The DMA then lands A
    with k on partitions natively ([128 k, 4 kb, 1024 m] tiles, 4KB
    descriptors), which deletes the entire device-side transpose
    pipeline of v1/v2: no PE transposes of A (512 instrs), no
    PSUM->SBUF at-copies (128 ACT instrs), no psum_tr/at pools.  PE per
    4MB chunk drops from ~80 instructions to 12, far below the DMA
    pace, so the stream is never gated by compute and the post-stream
    tail is tiny.
  - chunk q covers k in [512q, 512q+512) and ALL 1024 output rows; per
    k-128-block b one stationary h-tile serves BOTH output blocks
    (pacc0 += h^T prod[:,b,0:512], pacc1 += h^T prod[:,b,512:1024]).
  - adj+mask stream on the SP HWDGE ring in 1MB half-chunk dma_starts;
    triggers prefetched PREF chunks ahead; product adj*mask -> separate
    bf16 tile so adj AND mask slots free at the mul; h bf16;
    accumulation f32 in PSUM (rel err ~3e-3 vs the 2e-2 gate).
  - x loaded in natural 128-row blocks ("(j p) c" AP, 512B
    descriptors) on the scalar HWDGE ring; phase-0 computes h-tiles
    (PE transpose of x block + 128-wide bf16 matmul) one 1024-row
    group per two chunks, just ahead of use.
  - last chunk streams in 4 quarter-DMAs with per-quarter muls so the
    final matmuls start as early as possible; finalize(blk) transposes
    pacc back to row-major ([c,m] -> [m,c]) via PE, adds the self-loop
    h (x_own @ W^T, finalize-permuted), writes out with 2KB
    descriptors.
"""

import numpy as np
from contextlib import ExitStack

from concourse import bass, bacc, tile, mybir
from concourse import masks
from concourse.bass_utils import run_bass_kernel_spmd

N = 8192
C = 128
NCORES = 8
R = N // NCORES          # 1024 rows per core
M_BLK = 512              # psum accumulation block (free dim of main matmul)
NBLK = R // M_BLK        # 2 m-blocks per core
KB = 512                 # k-width per chunk
B = KB // 128            # 4 k-128-blocks per chunk
NCH = N // KB            # 16 chunks
KQ = 1024                # x phase-0 group size (rows)
XJ = KQ // 128           # 8 natural 128-row blocks per x group
JF = 4                   # finalize: rows per partition (out descriptor = JF*512B)
PREF = 3                 # chunks of DMA-trigger prefetch ahead of compute

F32 = mybir.dt.float32
BF16 = mybir.dt.bfloat16


def build_program():
    nc = bacc.Bacc("TRN2", target_bir_lowering=False, debug=False, num_devices=NCORES)

    adjT_d = nc.dram_tensor("adjT", [N, R], F32, kind="ExternalInput").ap()
    maskT_d = nc.dram_tensor("maskT", [N, R], F32, kind="ExternalInput").ap()
    x_d = nc.dram_tensor("x", [N, C], F32, kind="ExternalInput").ap()
    xo_d = nc.dram_tensor("x_own", [R, C], F32, kind="ExternalInput").ap()
    w_d = nc.dram_tensor("w", [C, C], F32, kind="ExternalInput").ap()
    out_d = nc.dram_tensor("out", [R, C], F32, kind="ExternalOutput").ap()

    with tile.TileContext(nc) as tc, ExitStack() as ctx:
        const_pool = ctx.enter_context(tc.tile_pool(name="const", bufs=1))
        xg_pool = ctx.enter_context(tc.tile_pool(name="xg", bufs=2))
        xt_pool = ctx.enter_context(tc.tile_pool(name="xt", bufs=3))
        h_pool = ctx.enter_context(tc.tile_pool(name="h", bufs=1))
        adj_pool = ctx.enter_context(tc.tile_pool(name="adj", bufs=4))
        mask_pool = ctx.enter_context(tc.tile_pool(name="mask", bufs=4))
        prod_pool = ctx.enter_context(tc.tile_pool(name="prod", bufs=3))
        fin_pool = ctx.enter_context(tc.tile_pool(name="fin", bufs=4))
        psum_acc = ctx.enter_context(tc.tile_pool(name="pacc", bufs=2, space="PSUM"))
        psum_misc = ctx.enter_context(tc.tile_pool(name="pmisc", bufs=2, space="PSUM"))
        psum_fin = ctx.enter_context(tc.tile_pool(name="pfin", bufs=1, space="PSUM"))

        ident = const_pool.tile([128, 128], F32)
        masks.make_identity(nc, ident[:])

        # ---- weight: W^T in bf16, loaded on the gpsimd queue ----
        w_sb = const_pool.tile([128, C], F32)
        nc.gpsimd.dma_start(out=w_sb[:], in_=w_d[:, :])
        psum_wt = psum_misc.tile([128, 128], F32, tag="pm")
        nc.tensor.transpose(psum_wt[:], w_sb[:], ident[:])
        wtr_sb = const_pool.tile([128, C], BF16)
        nc.vector.tensor_copy(wtr_sb[:], psum_wt[:])

        # ---- x loads: 8 groups of 1024 rows, natural 128-row blocks
        # (row = j*128 + p), 512B descriptors, on the scalar HWDGE ring
        xg_tiles = []
        for g in range(N // KQ):
            xg = xg_pool.tile([128, XJ, C], F32, tag="xg")
            nc.scalar.dma_start(
                out=xg[:],
                in_=x_d[g * KQ : (g + 1) * KQ, :].rearrange(
                    "(j p) c -> p j c", p=128, j=XJ
                ),
            )
            xg_tiles.append(xg)
        # x_own in the finalize permutation: row blk*512 + p*JF + j
        xo_sb = const_pool.tile([128, NBLK * JF, C], F32)
        for b in range(NBLK):
            nc.scalar.dma_start(
                out=xo_sb[:, b * JF : (b + 1) * JF, :],
                in_=xo_d[b * M_BLK : (b + 1) * M_BLK, :].rearrange(
                    "(p j) c -> p j c", p=128, j=JF
                ),
            )

        # h tile kg holds rows kg*128 + p (natural order), bf16
        h_sb = h_pool.tile([128, N // 128, C], BF16)
        ho_sb = const_pool.tile([128, NBLK * JF, C], F32)

        def h_tile_pipe(src_view, dst_view):
            psum_xt = psum_misc.tile([128, 128], F32, tag="pm")
            nc.tensor.transpose(psum_xt[:], src_view, ident[:])
            xt_sb = xt_pool.tile([128, 128], BF16)
            nc.scalar.copy(xt_sb[:], psum_xt[:])  # f32 -> bf16 rounds here
            psum_h = psum_misc.tile([128, 128], F32, tag="pm")
            nc.tensor.matmul(psum_h[:], xt_sb[:], wtr_sb[:], start=True, stop=True)
            nc.scalar.copy(dst_view, psum_h[:])

        def phase0_group(g):
            for j in range(XJ):
                h_tile_pipe(xg_tiles[g][:, j, :], h_sb[:, g * XJ + j, :])

        def phase0_own():
            # self-loop h in the finalize permutation (row blk*512 + JF*p + j)
            for j in range(NBLK * JF):
                h_tile_pipe(xo_sb[:, j, :], ho_sb[:, j, :])

        # ---- main loop ----
        def emit_triggers(q, parts):
            k0 = q * KB
            adj_t = adj_pool.tile([128, B, R], F32, tag="adj")
            mask_t = mask_pool.tile([128, B, R], F32, tag="mask")
            bw = B // parts          # k-128-blocks per part
            for hh in range(parts):
                bsl = slice(hh * bw, (hh + 1) * bw)
                rsl = slice(k0 + hh * bw * 128, k0 + (hh + 1) * bw * 128)
                nc.sync.dma_start(
                    out=adj_t[:, bsl, :],
                    in_=adjT_d[rsl, :].rearrange("(b p) m -> p b m", p=128),
                )
                nc.sync.dma_start(
                    out=mask_t[:, bsl, :],
                    in_=maskT_d[rsl, :].rearrange("(b p) m -> p b m", p=128),
                )
            return adj_t, mask_t

        def emit_muls(adj_t, mask_t, parts=2):
            # separate bf16 product tile: adj AND mask slots free at the mul
            prod_t = prod_pool.tile([128, B, R], BF16, tag="prod")
            bw = B // parts
            for hh in range(parts):
                bsl = slice(hh * bw, (hh + 1) * bw)
                nc.vector.tensor_mul(
                    prod_t[:, bsl, :], adj_t[:, bsl, :], mask_t[:, bsl, :]
                )
            return prod_t

        def emit_kbmms(q, prod_t, paccs):
            for b in range(B):
                kg = q * B + b
                st = kg == 0
                sp = kg == N // 128 - 1
                for blk in range(NBLK):
                    nc.tensor.matmul(
                        paccs[blk][:],
                        h_sb[:, kg, :],
                        prod_t[:, b, blk * M_BLK : (blk + 1) * M_BLK],
                        start=st,
                        stop=sp,
                    )

        def finalize(blk, pacc):
            # out rows blk*512 + JF*p + j; 2KB out descriptors
            psum_nat = psum_fin.tile([128, JF, C], F32)
            pacc_j = pacc[:].rearrange("p (m j) -> p j m", j=JF)
            for j in range(JF):
                otj = fin_pool.tile([128, 128], F32, tag="fin_t")
                nc.vector.tensor_copy(otj[:], pacc_j[:, j, :])
                nc.tensor.transpose(psum_nat[:, j, :], otj[:], ident[:])
            out_sb = fin_pool.tile([128, JF, C], F32, tag="fin_o")
            nc.vector.tensor_add(
                out_sb[:],
                psum_nat[:],
                ho_sb[:, blk * JF : (blk + 1) * JF, :],
            )
            r0 = blk * M_BLK
            nc.sync.dma_start(
                out=out_d[r0 : r0 + M_BLK, :].rearrange("(p j) c -> p j c", p=128),
                in_=out_sb[:],
            )

        last = NCH - 1
        paccs = {
            blk: psum_acc.tile([128, M_BLK], F32, name="pacc")
            for blk in range(NBLK)
        }
        trigs = {}
        for k in range(PREF):
            trigs[k] = emit_triggers(k, parts=2)
        for q in range(NCH):
            if q + PREF <= last:
                trigs[q + PREF] = emit_triggers(
                    q + PREF, parts=(4 if q + PREF == last else 2)
                )
            prod_t = emit_muls(*trigs.pop(q), parts=(4 if q == last else 2))
            if q % 2 == 0:
                phase0_group(q // 2)
            if q == 8:
                phase0_own()
            emit_kbmms(q, prod_t, paccs)
        finalize(0, paccs[0])
        finalize(1, paccs[1])

    nc.compile()
    return nc


_NC_CACHE = None


def _get_nc():
    global _NC_CACHE
    if _NC_CACHE is None:
        _NC_CACHE = build_program()
    return _NC_CACHE


def make_in_maps(x, adj, mask, W):
    x = np.ascontiguousarray(x, dtype=np.float32)
    W = np.ascontiguousarray(W, dtype=np.float32)
    in_maps = []
    for i in range(NCORES):
        r0 = i * R
        in_maps.append(
            {
                "adjT": np.ascontiguousarray(adj[r0 : r0 + R].T, dtype=np.float32),
                "maskT": np.ascontiguousarray(mask[r0 : r0 + R].T, dtype=np.float32),
                "x": x,
                "x_own": x[r0 : r0 + R],
                "w": W,
            }
        )
    return in_maps


def kernel(x, adj, mask, W):
    nc = _get_nc()
    in_maps = make_in_maps(x, adj, mask, W)
    res = run_bass_kernel_spmd(nc, in_maps, list(range(NCORES)))
    return np.concatenate([res.results[i]["out"] for i in range(NCORES)], axis=0)


# revision 8
# speedup vs baseline: 1.0456x; 1.0456x over previous
"""GCN layer kernel for Trainium2, 8-core row-parallel.

Computes out = (adj * mask + I) @ (x @ W^T) for N=8192, C_in=C_out=128.

Sharding: adj/mask row-blocks of 1024 across 8 cores; x, W replicated.
v4 (host-transposed A stream + host-transposed x):
  - each core's adj/mask row-slice is uploaded TRANSPOSED (adjT/maskT =
    [N, R] f32, a pure host-side layout choice).  The DMA lands A with
    k on partitions natively ([128 k, 4 kb, 1024 m] tiles, 4KB
    contiguous descriptors), deleting the entire device-side transpose
    pipeline of v1/v2 (512 PE transposes + 128 PSUM->SBUF copies).  PE
    per 4MB chunk is just 12 instructions, far below the DMA pace.
  - x is uploaded as xT = x.T ([C, N] f32) and loaded in 4 column
    chunks by ONE gpsimd (SWDGE) cast-DMA each (f32 -> bf16 inline,
    32KB contiguous source descriptors).  v3 loaded x row-blocks with
    512B descriptors on a HWDGE ring, which starved the adj stream at
    SDMA packet level every phase-0 group (~20us cadence dips to
    150-260 GB/s).  Phase-0 h-tiles are now one matmul each (stationary
    = xT column block, moving = W^T bf16), no x transposes, and the
    self-loop tiles come from strided xT APs so x_own is not uploaded.
  - chunk q covers k in [512q, 512q+512) and ALL 1024 output rows; per
    k-128-block b one stationary h-tile serves BOTH output blocks
    (pacc0 += h^T prod[:,b,0:512], pacc1 += h^T prod[:,b,512:1024]).
  - adj+mask stream on the SP HWDGE ring in 1MB half-chunk dma_starts;
    triggers prefetched PREF chunks ahead; product adj*mask -> separate
    bf16 tile so adj AND mask slots free at the mul; h bf16;
    accumulation f32 in PSUM (rel err ~3e-3 vs the 2e-2 gate).
  - last chunk streams in 4 quarter-DMAs with per-quarter muls;
    finalize(blk) transposes pacc back to row-major via PE, adds the
    self-loop h, writes out with 2KB descriptors.
"""

import numpy as np
from contextlib import ExitStack

from concourse import bass, bacc, tile, mybir
from concourse import masks
from concourse.bass_utils import run_bass_kernel_spmd

N = 8192
C = 128
NCORES = 8
R = N // NCORES          # 1024 rows per core
M_BLK = 512              # psum accumulation block (free dim of main matmul)
NBLK = R // M_BLK        # 2 m-blocks per core
KB = 512                 # k-width per chunk
B = KB // 128            # 4 k-128-blocks per chunk
NCH = N // KB            # 16 chunks
XP = 4                   # xT cast-DMA parts
JF = 4                   # finalize: rows per partition (out descriptor = JF*512B)
PREF = 3                 # chunks of DMA-trigger prefetch ahead of compute

F32 = mybir.dt.float32
BF16 = mybir.dt.bfloat16


def build_program():
    nc = bacc.Bacc("TRN2", target_bir_lowering=False, debug=False, num_devices=NCORES)

    adjT_d = nc.dram_tensor("adjT", [N, R], F32, kind="ExternalInput").ap()
    maskT_d = nc.dram_tensor("maskT", [N, R], F32, kind="ExternalInput").ap()
    xT_d = nc.dram_tensor("xT", [C, N], F32, kind="ExternalInput").ap()
    w_d = nc.dram_tensor("w", [C, C], F32, kind="ExternalInput").ap()
    out_d = nc.dram_tensor("out", [R, C], F32, kind="ExternalOutput").ap()

    with tile.TileContext(nc) as tc, ExitStack() as ctx:
        const_pool = ctx.enter_context(tc.tile_pool(name="const", bufs=1))
        h_pool = ctx.enter_context(tc.tile_pool(name="h", bufs=1))
        adj_pool = ctx.enter_context(tc.tile_pool(name="adj", bufs=4))
        mask_pool = ctx.enter_context(tc.tile_pool(name="mask", bufs=4))
        prod_pool = ctx.enter_context(tc.tile_pool(name="prod", bufs=3))
        fin_pool = ctx.enter_context(tc.tile_pool(name="fin", bufs=4))
        psum_acc = ctx.enter_context(tc.tile_pool(name="pacc", bufs=2, space="PSUM"))
        psum_misc = ctx.enter_context(tc.tile_pool(name="pmisc", bufs=2, space="PSUM"))
        psum_fin = ctx.enter_context(tc.tile_pool(name="pfin", bufs=1, space="PSUM"))

        ident = const_pool.tile([128, 128], F32)
        masks.make_identity(nc, ident[:])

        # ---- weight: W^T in bf16, loaded on the gpsimd queue ----
        w_sb = const_pool.tile([128, C], F32)
        nc.gpsimd.dma_start(out=w_sb[:], in_=w_d[:, :])
        psum_wt = psum_misc.tile([128, 128], F32, tag="pm")
        nc.tensor.transpose(psum_wt[:], w_sb[:], ident[:])
        wtr_sb = const_pool.tile([128, C], BF16)
        nc.vector.tensor_copy(wtr_sb[:], psum_wt[:])

        # ---- xT: [c, n] bf16 via SWDGE cast-DMA, 4 column parts ----
        xTb = const_pool.tile([128, N], BF16)
        for p in range(XP):
            csl = slice(p * (N // XP), (p + 1) * (N // XP))
            nc.gpsimd.dma_start(out=xTb[:, csl], in_=xT_d[:, csl])

        # h tile kg holds rows kg*128 + p (natural order), bf16
        h_sb = h_pool.tile([128, N // 128, C], BF16)
        ho_sb = const_pool.tile([128, NBLK * JF, C], BF16)

        def h_tile_pipe(stat_view, dst_view):
            # h-block = (xT cols)^T @ W^T; stationary = xT column block
            psum_h = psum_misc.tile([128, 128], F32, tag="pm")
            nc.tensor.matmul(psum_h[:], stat_view, wtr_sb[:], start=True, stop=True)
            nc.scalar.copy(dst_view, psum_h[:])

        def phase0_group(g):
            # h tiles for x rows [g*1024, (g+1)*1024)
            for j in range(8):
                kg = g * 8 + j
                h_tile_pipe(
                    xTb[:, kg * 128 : (kg + 1) * 128], h_sb[:, kg, :]
                )

        def phase0_own():
            # self-loop h in the finalize permutation (row blk*512 + JF*p + j).
            # The k-axis is rotated per-core on the host so this core's own
            # rows are xT columns [0, R): a fixed range in the SPMD program.
            for blk in range(NBLK):
                v = xTb[:, blk * M_BLK : (blk + 1) * M_BLK].rearrange(
                    "p (m j) -> p j m", j=JF
                )
                for j in range(JF):
                    h_tile_pipe(v[:, j, :], ho_sb[:, blk * JF + j, :])

        # ---- main loop ----
        def emit_triggers(q, parts):
            k0 = q * KB
            adj_t = adj_pool.tile([128, B, R], F32, tag="adj")
            mask_t = mask_pool.tile([128, B, R], F32, tag="mask")
            bw = B // parts          # k-128-blocks per part
            for hh in range(parts):
                bsl = slice(hh * bw, (hh + 1) * bw)
                rsl = slice(k0 + hh * bw * 128, k0 + (hh + 1) * bw * 128)
                nc.sync.dma_start(
                    out=adj_t[:, bsl, :],
                    in_=adjT_d[rsl, :].rearrange("(b p) m -> p b m", p=128),
                )
                nc.sync.dma_start(
                    out=mask_t[:, bsl, :],
                    in_=maskT_d[rsl, :].rearrange("(b p) m -> p b m", p=128),
                )
            return adj_t, mask_t

        def emit_muls(adj_t, mask_t, parts=2):
            # separate bf16 product tile: adj AND mask slots free at the mul
            prod_t = prod_pool.tile([128, B, R], BF16, tag="prod")
            bw = B // parts
            for hh in range(parts):
                bsl = slice(hh * bw, (hh + 1) * bw)
                nc.vector.tensor_mul(
                    prod_t[:, bsl, :], adj_t[:, bsl, :], mask_t[:, bsl, :]
                )
            return prod_t

        def emit_kbmms(q, prod_t, paccs):
            for b in range(B):
                kg = q * B + b
                st = kg == 0
                sp = kg == N // 128 - 1
                for blk in range(NBLK):
                    nc.tensor.matmul(
                        paccs[blk][:],
                        h_sb[:, kg, :],
                        prod_t[:, b, blk * M_BLK : (blk + 1) * M_BLK],
                        start=st,
                        stop=sp,
                    )

        def finalize(blk, pacc):
            # out rows blk*512 + JF*p + j; 2KB out descriptors
            psum_nat = psum_fin.tile([128, JF, C], F32)
            pacc_j = pacc[:].rearrange("p (m j) -> p j m", j=JF)
            for j in range(JF):
                otj = fin_pool.tile([128, 128], F32, tag="fin_t")
                nc.vector.tensor_copy(otj[:], pacc_j[:, j, :])
                nc.tensor.transpose(psum_nat[:, j, :], otj[:], ident[:])
            out_sb = fin_pool.tile([128, JF, C], F32, tag="fin_o")
            nc.vector.tensor_add(
                out_sb[:],
                psum_nat[:],
                ho_sb[:, blk * JF : (blk + 1) * JF, :],
            )
            r0 = blk * M_BLK
            nc.sync.dma_start(
                out=out_d[r0 : r0 + M_BLK, :].rearrange("(p j) c -> p j c", p=128),
                in_=out_sb[:],
            )

        last = NCH - 1
        paccs = {
            blk: psum_acc.tile([128, M_BLK], F32, name="pacc")
            for blk in range(NBLK)
        }
        trigs = {}
        for k in range(PREF):
            trigs[k] = emit_triggers(k, parts=2)
        for q in range(NCH):
            if q + PREF <= last:
                trigs[q + PREF] = emit_triggers(
                    q + PREF, parts=(4 if q + PREF == last else 2)
                )
            prod_t = emit_muls(*trigs.pop(q), parts=(4 if q == last else 2))
            if q % 2 == 0:
                phase0_group(q // 2)
            if q == 8:
                phase0_own()
            emit_kbmms(q, prod_t, paccs)
        finalize(0, paccs[0])
        finalize(1, paccs[1])

    nc.compile()
    return nc


_NC_CACHE = None


def _get_nc():
    global _NC_CACHE
    if _NC_CACHE is None:
        _NC_CACHE = build_program()
    return _NC_CACHE


def _rolled_T(a, r0):
    # ascontiguousarray(np.roll(a, -r0, axis=1).T) in one transpose-copy:
    # row k' of the result is column (r0 + k') % N of `a`
    n = a.shape[1]
    out = np.empty((n, a.shape[0]), dtype=np.float32)
    out[: n - r0] = a[:, r0:].T
    out[n - r0 :] = a[:, :r0].T
    return out


def make_in_maps(x, adj, mask, W):
    W = np.ascontiguousarray(W, dtype=np.float32)
    in_maps = []
    for i in range(NCORES):
        r0 = i * R
        # the k-axis is rotated by r0 per core (pure accumulation-order
        # change) so each core's own rows sit at k' in [0, R) -- the
        # SPMD-shared program can then address the self-loop x block at
        # a fixed location.  adjT/maskT/xT all use the same rotation.
        in_maps.append(
            {
                "adjT": _rolled_T(adj[r0 : r0 + R], r0),
                "maskT": _rolled_T(mask[r0 : r0 + R], r0),
                "xT": np.ascontiguousarray(_rolled_T(x.T, r0).T),
                "w": W,
            }
        )
    return in_maps


def kernel(x, adj, mask, W):
    nc = _get_nc()
    in_maps = make_in_maps(x, adj, mask, W)
    res = run_bass_kernel_spmd(nc, in_maps, list(range(NCORES)))
    return np.concatenate([res.results[i]["out"] for i in range(NCORES)], axis=0)


# revision 9
# speedup vs baseline: 1.2512x; 1.1966x over previous
"""GCN layer kernel for Trainium2, 8-core row-parallel.

Computes out = (adj * mask + I) @ (x @ W^T) for N=8192, C_in=C_out=128.

Sharding: adj/mask row-blocks of 1024 across 8 cores; x, W replicated.
v5 (SWDGE bf16-cast stream):
  - each core's adj/mask row-slice is uploaded TRANSPOSED (adjT/maskT =
    [N, R] f32, a pure host-side layout choice).  The DMA lands A with
    k on partitions natively, deleting the entire device-side transpose
    pipeline of v1/v2.  PE per 4MB chunk is just 12 instructions.
  - the adj/mask stream rides the gpsimd SWDGE queue with inline
    f32->bf16 cast (4KB f32 source descriptors, bf16 SBUF tiles): muls
    are all-bf16 (2 elem/cyc on DVE), SBUF tiles halve so pools go 6
    deep, and DVE can never gate the stream even at full stack rate --
    the post-stream tail is one half-chunk mul + 4 matmuls + finalize.
  - xT = x.T ([C, N] f32) loads on the otherwise-idle SP HWDGE ring
    (32KB/partition contiguous descriptors, 2 parts), so the main
    stream starts at t~1us; phase-0 h-tiles are one f32 matmul each
    (stationary = xT column block), hoisted early by the scheduler.
  - the k-axis is rotated by r0 per core (host-side, pure accumulation
    order change) so each core's self-loop x block sits at a fixed xT
    column range for the SPMD-shared program.
  - chunk q covers k in [512q, 512q+512) and ALL 1024 output rows; per
    k-128-block b one stationary h-tile serves BOTH output blocks.
  - accumulation f32 in PSUM; rel err ~4e-3 vs the 2e-2 gate (adj and
    mask are bf16-rounded before the product).
  - last chunk streams in 4 quarter-DMAs with per-quarter muls;
    finalize transposes pacc back to row-major via PE, adds the
    self-loop h, writes out with 2KB descriptors on the SP ring.
"""

import numpy as np
from contextlib import ExitStack

from concourse import bass, bacc, tile, mybir
from concourse import masks
from concourse.bass_utils import run_bass_kernel_spmd

N = 8192
C = 128
NCORES = 8
R = N // NCORES          # 1024 rows per core
M_BLK = 512              # psum accumulation block (free dim of main matmul)
NBLK = R // M_BLK        # 2 m-blocks per core
KB = 512                 # k-width per chunk
B = KB // 128            # 4 k-128-blocks per chunk
NCH = N // KB            # 16 chunks
JF = 4                   # finalize: rows per partition (out descriptor = JF*512B)
PREF = 4                 # chunks of DMA-trigger prefetch ahead of compute

F32 = mybir.dt.float32
BF16 = mybir.dt.bfloat16


def build_program():
    nc = bacc.Bacc("TRN2", target_bir_lowering=False, debug=False, num_devices=NCORES)

    adjT_d = nc.dram_tensor("adjT", [N, R], F32, kind="ExternalInput").ap()
    maskT_d = nc.dram_tensor("maskT", [N, R], F32, kind="ExternalInput").ap()
    xT_d = nc.dram_tensor("xT", [C, N], F32, kind="ExternalInput").ap()
    w_d = nc.dram_tensor("w", [C, C], F32, kind="ExternalInput").ap()
    out_d = nc.dram_tensor("out", [R, C], F32, kind="ExternalOutput").ap()

    with tile.TileContext(nc) as tc, ExitStack() as ctx:
        const_pool = ctx.enter_context(tc.tile_pool(name="const", bufs=1))
        h_pool = ctx.enter_context(tc.tile_pool(name="h", bufs=1))
        adj_pool = ctx.enter_context(tc.tile_pool(name="adj", bufs=6))
        mask_pool = ctx.enter_context(tc.tile_pool(name="mask", bufs=6))
        prod_pool = ctx.enter_context(tc.tile_pool(name="prod", bufs=4))
        fin_pool = ctx.enter_context(tc.tile_pool(name="fin", bufs=4))
        psum_acc = ctx.enter_context(tc.tile_pool(name="pacc", bufs=2, space="PSUM"))
        psum_misc = ctx.enter_context(tc.tile_pool(name="pmisc", bufs=3, space="PSUM"))
        psum_fin = ctx.enter_context(tc.tile_pool(name="pfin", bufs=1, space="PSUM"))

        # ---- xT on the SP HWDGE ring: 2 parts, 16KB/partition each ----
        xTf = const_pool.tile([128, N], F32)
        for p in range(2):
            csl = slice(p * (N // 2), (p + 1) * (N // 2))
            nc.sync.dma_start(out=xTf[:, csl], in_=xT_d[:, csl])

        ident = const_pool.tile([128, 128], F32)
        masks.make_identity(nc, ident[:])

        # ---- weight: W^T in f32 (phase-0 matmuls are all-f32) ----
        w_sb = const_pool.tile([128, C], F32)
        nc.gpsimd.dma_start(out=w_sb[:], in_=w_d[:, :])
        psum_wt = psum_misc.tile([128, 128], F32, tag="pm")
        nc.tensor.transpose(psum_wt[:], w_sb[:], ident[:])
        wtr_sb = const_pool.tile([128, C], F32)
        nc.vector.tensor_copy(wtr_sb[:], psum_wt[:])

        # h tile kg holds rows kg*128 + p (natural order), bf16
        h_sb = h_pool.tile([128, N // 128, C], BF16)
        ho_sb = const_pool.tile([128, NBLK * JF, C], BF16)

        def h_tile_pipe(stat_view, dst_view):
            # h-block = (xT cols)^T @ W^T; stationary = xT column block
            psum_h = psum_misc.tile([128, 128], F32, tag="pm")
            nc.tensor.matmul(psum_h[:], stat_view, wtr_sb[:], start=True, stop=True)
            nc.scalar.copy(dst_view, psum_h[:])

        def phase0_group(g):
            # h tiles for x rows [g*1024, (g+1)*1024)
            for j in range(8):
                kg = g * 8 + j
                h_tile_pipe(
                    xTf[:, kg * 128 : (kg + 1) * 128], h_sb[:, kg, :]
                )

        def phase0_own():
            # self-loop h in the finalize permutation (row blk*512 + JF*p + j).
            # The k-axis is rotated per-core on the host so this core's own
            # rows are xT columns [0, R): a fixed range in the SPMD program.
            for blk in range(NBLK):
                v = xTf[:, blk * M_BLK : (blk + 1) * M_BLK].rearrange(
                    "p (m j) -> p j m", j=JF
                )
                for j in range(JF):
                    h_tile_pipe(v[:, j, :], ho_sb[:, blk * JF + j, :])

        # ---- main loop ----
        def emit_triggers(q, parts):
            k0 = q * KB
            adj_t = adj_pool.tile([128, B, R], BF16, tag="adj")
            mask_t = mask_pool.tile([128, B, R], BF16, tag="mask")
            bw = B // parts          # k-128-blocks per part
            for hh in range(parts):
                bsl = slice(hh * bw, (hh + 1) * bw)
                rsl = slice(k0 + hh * bw * 128, k0 + (hh + 1) * bw * 128)
                nc.gpsimd.dma_start(
                    out=adj_t[:, bsl, :],
                    in_=adjT_d[rsl, :].rearrange("(b p) m -> p b m", p=128),
                )
                nc.gpsimd.dma_start(
                    out=mask_t[:, bsl, :],
                    in_=maskT_d[rsl, :].rearrange("(b p) m -> p b m", p=128),
                )
            return adj_t, mask_t

        def emit_muls(adj_t, mask_t, parts=2):
            # separate bf16 product tile: adj AND mask slots free at the mul
            prod_t = prod_pool.tile([128, B, R], BF16, tag="prod")
            bw = B // parts
            for hh in range(parts):
                bsl = slice(hh * bw, (hh + 1) * bw)
                nc.vector.tensor_mul(
                    prod_t[:, bsl, :], adj_t[:, bsl, :], mask_t[:, bsl, :]
                )
            return prod_t

        def emit_kbmms(q, prod_t, paccs):
            for b in range(B):
                kg = q * B + b
                st = kg == 0
                sp = kg == N // 128 - 1
                for blk in range(NBLK):
                    nc.tensor.matmul(
                        paccs[blk][:],
                        h_sb[:, kg, :],
                        prod_t[:, b, blk * M_BLK : (blk + 1) * M_BLK],
                        start=st,
                        stop=sp,
                    )

        def finalize(blk, pacc):
            # out rows blk*512 + JF*p + j; 2KB out descriptors
            psum_nat = psum_fin.tile([128, JF, C], F32)
            pacc_j = pacc[:].rearrange("p (m j) -> p j m", j=JF)
            for j in range(JF):
                otj = fin_pool.tile([128, 128], F32, tag="fin_t")
                nc.vector.tensor_copy(otj[:], pacc_j[:, j, :])
                nc.tensor.transpose(psum_nat[:, j, :], otj[:], ident[:])
            out_sb = fin_pool.tile([128, JF, C], F32, tag="fin_o")
            nc.vector.tensor_add(
                out_sb[:],
                psum_nat[:],
                ho_sb[:, blk * JF : (blk + 1) * JF, :],
            )
            r0 = blk * M_BLK
            nc.sync.dma_start(
                out=out_d[r0 : r0 + M_BLK, :].rearrange("(p j) c -> p j c", p=128),
                in_=out_sb[:],
            )

        last = NCH - 1
        paccs = {
            blk: psum_acc.tile([128, M_BLK], F32, name="pacc")
            for blk in range(NBLK)
        }
        trigs = {}
        for k in range(PREF):
            trigs[k] = emit_triggers(k, parts=2)
        for q in range(NCH):
            if q + PREF <= last:
                trigs[q + PREF] = emit_triggers(
                    q + PREF, parts=(4 if q + PREF == last else 2)
                )
            prod_t = emit_muls(*trigs.pop(q), parts=(4 if q == last else 2))
            if q % 2 == 0:
                phase0_group(q // 2)
            if q == 8:
                phase0_own()
            emit_kbmms(q, prod_t, paccs)
        finalize(0, paccs[0])
        finalize(1, paccs[1])

    nc.compile()
    return nc


_NC_CACHE = None


def _get_nc():
    global _NC_CACHE
    if _NC_CACHE is None:
        _NC_CACHE = build_program()
    return _NC_CACHE


def _rolled_T(a, r0):
    # ascontiguousarray(np.roll(a, -r0, axis=1).T) in one transpose-copy:
    # row k' of the result is column (r0 + k') % N of `a`
    n = a.shape[1]
    out = np.empty((n, a.shape[0]), dtype=np.float32)
    out[: n - r0] = a[:, r0:].T
    out[n - r0 :] = a[:, :r0].T
    return out


def make_in_maps(x, adj, mask, W):
    W = np.ascontiguousarray(W, dtype=np.float32)
    in_maps = []
    for i in range(NCORES):
        r0 = i * R
        # the k-axis is rotated by r0 per core (pure accumulation-order
        # change) so each core's own rows sit at k' in [0, R) -- the
        # SPMD-shared program can then address the self-loop x block at
        # a fixed location.  adjT/maskT/xT all use the same rotation.
        in_maps.append(
            {
                "adjT": _rolled_T(adj[r0 : r0 + R], r0),
                "maskT": _rolled_T(mask[r0 : r0 + R], r0),
                "xT": np.ascontiguousarray(_rolled_T(x.T, r0).T),
                "w": W,
            }
        )
    return in_maps


def kernel(x, adj, mask, W):
    nc = _get_nc()
    in_maps = make_in_maps(x, adj, mask, W)
    res = run_bass_kernel_spmd(nc, in_maps, list(range(NCORES)))
    return np.concatenate([res.results[i]["out"] for i in range(NCORES)], axis=0)
